# revision 29
# baseline (speedup 1.0000x reference)
"""Block-sparse 3-layer MLP on 8 Trainium2 NeuronCores.

Reference computation (fp32):
    h1 = relu(x @ (W1*expand(mask1)).T + b1)       x:[B,2048] W1:[4096,2048]
    h2 = relu(h1 @ (W2*expand(mask2)).T + b2)      W2:[4096,4096]
    out = h2 @ Wo.T + bo                           Wo:[1024,4096] -> [B,1024]

Strategy: data-parallel over the batch (B=8192 -> 1024 rows/core), no
collectives. Masks are applied to the weights on the host (free) and all
matmuls run dense on the PE array: at density 0.5 with 32x32 mask tiles,
skipping zero tiles via PE-array tiling is slower than dense (packed 32x32
tiles reach only ~36% of dense PE throughput), and fp8 DoubleRow (2x PE
rate) fails the 2e-2 error gate (e4m3 one-pass ~6% rel err; an accurate
3-term hi/lo split needs 1.5x the products, i.e. slower than bf16).

All matmul operands are bf16: same 1 cycle/row PE rate as f32r but half
the DMA bytes and SBUF footprint (rel err ~4e-3, budget 2e-2).
Activations are feature-major [features, batch] so biases are
per-partition and lhsT panels are host-pretransposed [128, K] blocks.

Per core, three phases, PE-saturated throughout:
  L1: 32 m-tiles, psum-accumulated over 16 k-tiles, RELU -> h1 resident
      in SBUF (bf16, 8MB). Inputs stream on the three DMA-capable rings
      (sync/gpsimd/scalar) one descriptor per k-tile in PE consumption
      order, critical tiles at the ring heads.
  L2: 32 m-tiles over 32 k-tiles, RELU -> h2 resident (8MB). ps2 opens
      alongside ps1 (4+4 psum banks) so its first accumulation does not
      wait on L1's last psum release.
  L3: output accumulated directly in PSUM: 2 groups of 4 output m-tiles,
      each group's 4 accumulators [128,1024] = 8 psum banks, k-outer over
      the 32 h2 tiles. No SBUF accumulator and no vector adds; the tail
      is the last bias-activation (split scalar/vector) + output DMA
      spread over all three rings.
"""

import sys

sys.path.insert(0, "/opt/trn_rl_repo")

import numpy as np

from concourse import bacc, mybir, tile
from concourse.bass_utils import run_bass_kernel_spmd

F32 = mybir.dt.float32
BF16 = mybir.dt.bfloat16
RELU = mybir.ActivationFunctionType.Relu
IDENT = mybir.ActivationFunctionType.Identity

N_CORES = 8
TILE = 32  # block-sparse tile size of the masks
P = 128  # partitions


def _build(nc, d_in, d_h, d_out, bc):
    """Emit the per-core kernel. bc = batch columns per core."""
    kt1 = d_in // P  # k-tiles in layer 1 (16)
    mt1 = d_h // P  # m-tiles of h1 == k-tiles of layer 2 (32)
    mt2 = d_h // P  # m-tiles of h2 == k-tiles of layer 3 (32)
    mot = d_out // P  # m-tiles of out (8)
    sw = min(512, bc)  # psum strip width
    ns = bc // sw  # strips per row of tiles
    GW = 4  # output m-tiles per L3 psum group (4 x 2 banks = 8 banks)

    xt_d = nc.dram_tensor("xt", [kt1, P, bc], BF16, kind="ExternalInput")
    w1_d = nc.dram_tensor("w1", [mt1, P, d_in], BF16, kind="ExternalInput")
    b1_d = nc.dram_tensor("b1", [P, mt1], F32, kind="ExternalInput")
    w2_d = nc.dram_tensor("w2", [mt2, P, d_h], BF16, kind="ExternalInput")
    b2_d = nc.dram_tensor("b2", [P, mt2], F32, kind="ExternalInput")
    wo_d = nc.dram_tensor("wo", [mt2, P, d_out], BF16, kind="ExternalInput")
    bo_d = nc.dram_tensor("bo", [P, mot], F32, kind="ExternalInput")
    out_d = nc.dram_tensor("out", [mot, P, bc], F32, kind="ExternalOutput")

    with tile.TileContext(nc) as tc:
        with (
            tc.tile_pool(name="bias", bufs=1) as bias_pool,
            tc.tile_pool(name="h1", bufs=1) as h1_pool,
            tc.tile_pool(name="h2", bufs=1) as h2_pool,
            tc.tile_pool(name="w2p", bufs=3) as w2_pool,
        ):
            b1_sb = bias_pool.tile([P, mt1], F32, tag="b1")
            b2_sb = bias_pool.tile([P, mt2], F32, tag="b2")
            bo_sb = bias_pool.tile([P, mot], F32, tag="bo")

            h1 = []
            h2 = []
            wo_pre = {}
            if True:
                # ---------------- Layer 1 ----------------
                with (
                    tc.tile_pool(name="xtp", bufs=1) as xt_pool,
                    tc.tile_pool(name="w1p", bufs=4) as w1_pool,
                    tc.tile_pool(name="ps1", bufs=3, space="PSUM") as ps1_pool,
                    tc.tile_pool(name="wrm", bufs=1, space="PSUM") as wrm_pool,
                ):
                    # The tile scheduler batches DMA-completion semaphores
                    # per ring segment between consumers, so loads must be
                    # EMITTED interleaved with the matmuls that consume them
                    # or the first matmul waits on a whole batch. Each xt
                    # k-tile is its own SBUF tile written by one descriptor
                    # (slices of a shared tile create cross-ring
                    # write-ordering semaphores); loads pace 3 k-tiles ahead
                    # of consumption, W1 panels 2 ahead, biases on the
                    # scalar ring where slack allows.
                    rings = [nc.sync, nc.gpsimd, nc.scalar]
                    xt = [
                        xt_pool.tile([P, bc], BF16, name=f"xt_{kt}", tag=f"xt_{kt}")
                        for kt in range(kt1)
                    ]

                    def load_xt(kt, eng):
                        eng.dma_start(out=xt[kt][:], in_=xt_d[kt])

                    # critical head on the two HW-DGE rings (sync, scalar);
                    # gpsimd's SW-DGE completion signals are coalesced and
                    # release late, so it only carries loads with slack
                    load_xt(0, nc.scalar)
                    w1pre = {}
                    w2pre = []
                    # PE warm-up: dummy matmuls on a memset tile run while
                    # the input DMAs stage, burning the DVFS p-state ramp
                    # (0.65 -> 1.2 -> 2.4 GHz takes ~3us of continuous PE
                    # busy) so the real matmuls start at full clock.
                    warm = xt_pool.tile([P, sw], BF16, tag="warm")
                    nc.vector.memset(warm[:], 0.0)
                    wps = wrm_pool.tile([P, sw], F32, tag="wps")
                    for i in range(5):
                        nc.tensor.matmul(
                            wps[:],
                            warm[:, 0:P],
                            warm[:],
                            start=(i == 0),
                            stop=(i == 4),
                        )
                    w1t0 = w1_pool.tile([P, d_in], BF16, tag="w1t")
                    nc.sync.dma_start(out=w1t0[:], in_=w1_d[0])
                    w1pre[0] = w1t0
                    load_xt(1, nc.sync)
                    nc.scalar.dma_start(out=b1_sb[:], in_=b1_d[:])
                    load_xt(2, nc.scalar)
                    load_xt(3, nc.sync)
                    xt_next = 4
                    w1_next = 1

                    def issue_w1(mt):
                        t = w1_pool.tile([P, d_in], BF16, tag="w1t")
                        eng = nc.sync if mt == 1 else rings[mt % 3]
                        eng.dma_start(out=t[:], in_=w1_d[mt])
                        w1pre[mt] = t

                    for mt in range(mt1):
                        while mt > 0 and w1_next <= min(mt + 2, mt1 - 1):
                            issue_w1(w1_next)
                            w1_next += 1
                        w1t = w1pre.pop(mt)
                        ps = ps1_pool.tile([P, bc], F32, tag="ps1")
                        for kt in range(kt1):
                            if mt == 0:
                                while xt_next <= min(kt + 3, kt1 - 1):
                                    if xt_next < 8:
                                        eng = nc.sync if xt_next % 2 else nc.scalar
                                    else:
                                        eng = rings[xt_next % 3]
                                    load_xt(xt_next, eng)
                                    xt_next += 1
                                if kt == 6:
                                    # W1 panels 1,2 once xt's head is in
                                    while w1_next <= 2:
                                        issue_w1(w1_next)
                                        w1_next += 1
                            for n in range(ns):
                                nc.tensor.matmul(
                                    ps[:, n * sw : (n + 1) * sw],
                                    w1t[:, kt * P : (kt + 1) * P],
                                    xt[kt][:, n * sw : (n + 1) * sw],
                                    start=(kt == 0),
                                    stop=(kt == kt1 - 1),
                                )
                        if mt == 0:
                            nc.scalar.dma_start(out=b2_sb[:], in_=b2_d[:])
                            nc.scalar.dma_start(out=bo_sb[:], in_=bo_d[:])
                        if mt == 3:
                            # W2 panels 0,1 on the now-idle scalar ring
                            for i in range(2):
                                t = w2_pool.tile([P, d_h], BF16, tag="w2t")
                                nc.scalar.dma_start(out=t[:], in_=w2_d[i])
                                w2pre.append(t)
                        h = h1_pool.tile(
                            [P, bc], BF16, name=f"h1_{mt}", tag=f"h1_{mt}"
                        )
                        nc.scalar.activation(
                            h[:], ps[:], RELU, bias=b1_sb[:, mt : mt + 1]
                        )
                        h1.append(h)

                # ---------------- Layer 2 ----------------
                es_wop = tc.tile_pool(name="wop", bufs=10)
                wo_pool = es_wop.__enter__()
                es_ps2 = tc.tile_pool(name="ps2", bufs=3, space="PSUM")
                ps2_pool = es_ps2.__enter__()
                w2map = {0: w2pre[0], 1: w2pre[1]}
                w2_next = 2

                def issue_w2(mt):
                    t = w2_pool.tile([P, d_h], BF16, tag="w2t")
                    eng = nc.sync if mt % 2 else nc.gpsimd
                    eng.dma_start(out=t[:], in_=w2_d[mt])
                    w2map[mt] = t

                for mt in range(mt2):
                    while w2_next <= min(mt + 1, mt2 - 1):
                        issue_w2(w2_next)
                        w2_next += 1
                    w2t = w2map.pop(mt)
                    ps = ps2_pool.tile([P, bc], F32, tag="ps2")
                    for kt in range(mt1):
                        for n in range(ns):
                            nc.tensor.matmul(
                                ps[:, n * sw : (n + 1) * sw],
                                w2t[:, kt * P : (kt + 1) * P],
                                h1[kt][:, n * sw : (n + 1) * sw],
                                start=(kt == 0),
                                stop=(kt == mt1 - 1),
                            )
                    h = h2_pool.tile([P, bc], BF16, name=f"h2_{mt}", tag=f"h2_{mt}")
                    nc.scalar.activation(h[:], ps[:], RELU, bias=b2_sb[:, mt : mt + 1])
                    h2.append(h)
                    if mt == 24:
                        # L3's first half-panels of Wo on the mostly-idle
                        # scalar ring, well ahead of L3's start
                        for kt in range(4):
                            t = wo_pool.tile([P, GW * P], BF16, tag="wot")
                            nc.scalar.dma_start(out=t[:], in_=wo_d[kt][:, 0 : GW * P])
                            wo_pre[kt] = t

            es_ps2.__exit__(None, None, None)

            # ---------------- Layer 3 ----------------
            # Output accumulates in PSUM across all 32 k-tiles: per group
            # of GW=4 output m-tiles, 4 accumulators of [128, bc] f32
            # occupy all 8 psum banks; Wo half-panels stream k-outer.
            with (
                tc.tile_pool(name="ps3", bufs=1, space="PSUM") as ps3_pool,
                tc.tile_pool(name="osb", bufs=GW) as osb_pool,
            ):
                seq = [(g, kt) for g in range(mot // GW) for kt in range(mt2)]
                wot_map = {(0, kt): t for kt, t in wo_pre.items()}
                wo_next = 0

                def issue_wo(g, kt):
                    if (g, kt) in wot_map:
                        return
                    t = wo_pool.tile([P, GW * P], BF16, tag="wot")
                    eng = nc.sync if kt % 2 else nc.gpsimd
                    eng.dma_start(
                        out=t[:], in_=wo_d[kt][:, g * GW * P : (g + 1) * GW * P]
                    )
                    wot_map[(g, kt)] = t

                for idx, (g, kt) in enumerate(seq):
                    if kt == 0:
                        pss = [
                            ps3_pool.tile(
                                [P, bc], F32, name=f"ps3_{j}", tag=f"ps3_{j}"
                            )
                            for j in range(GW)
                        ]
                    while wo_next <= min(idx + 4, len(seq) - 1):
                        issue_wo(*seq[wo_next])
                        wo_next += 1
                    wot = wot_map.pop((g, kt))
                    for j in range(GW):
                        for n in range(ns):
                            nc.tensor.matmul(
                                pss[j][:, n * sw : (n + 1) * sw],
                                wot[:, j * P : (j + 1) * P],
                                h2[kt][:, n * sw : (n + 1) * sw],
                                start=(kt == 0),
                                stop=(kt == mt2 - 1),
                            )
                    if kt != mt2 - 1:
                        continue
                    last_g = g == mot // GW - 1
                    # finalize all accumulators first (scalar half / vector
                    # half per j), then issue the output DMAs on the two
                    # HW-DGE rings only -- gpsimd SW-DGE copies would hold
                    # the final drain hostage, and DMA issues interleaved on
                    # the scalar queue would delay the activations.
                    osbs = []
                    for j in range(GW):
                        mo = g * GW + j
                        osb = osb_pool.tile([P, bc], F32, tag="osb")
                        if j == 0 and not last_g:
                            # single full-width op releases this psum
                            # buffer fastest for the next group
                            nc.scalar.activation(
                                osb[:], pss[j][:], IDENT, bias=bo_sb[:, mo : mo + 1]
                            )
                        else:
                            nc.scalar.activation(
                                osb[:, 0:sw],
                                pss[j][:, 0:sw],
                                IDENT,
                                bias=bo_sb[:, mo : mo + 1],
                            )
                            nc.vector.tensor_scalar_add(
                                osb[:, sw:], pss[j][:, sw:], bo_sb[:, mo : mo + 1]
                            )
                        osbs.append(osb)
                    for j in range(GW):
                        mo = g * GW + j
                        osb = osbs[j]
                        nc.sync.dma_start(out=out_d[mo][:, 0:sw], in_=osb[:, 0:sw])
                        nc.scalar.dma_start(out=out_d[mo][:, sw:], in_=osb[:, sw:])

            es_wop.__exit__(None, None, None)

    nc.compile()
    return nc


def _expand_mask(mask, t=TILE):
    return np.repeat(np.repeat(np.asarray(mask, dtype=bool), t, axis=0), t, axis=1)


def _pack_lhsT(w, d_m, d_k):
    """[d_m, d_k] weights -> [d_m/P, P, d_k] panels.

    panel[mt, i, kt*P + j] = w[mt*P + j, kt*P + i], so each [P, P] slice of a
    panel is a ready-to-use lhsT block (partition dim = contraction dim).
    """
    mt, kt = d_m // P, d_k // P
    return np.ascontiguousarray(
        w.reshape(mt, P, kt, P).transpose(0, 3, 2, 1).reshape(mt, P, d_k)
    )


def _pack_out_panels(w, d_m, d_k):
    """[d_m, d_k] weights -> [d_k/P, P, d_m] panels keyed by the k-tile.

    panel[kt, i, mo*P + j] = w[mo*P + j, kt*P + i].
    """
    mt, kt = d_m // P, d_k // P
    return np.ascontiguousarray(
        w.reshape(mt, P, kt, P).transpose(2, 3, 0, 1).reshape(kt, P, d_m)
    )


def _pack_bias(b):
    n = b.shape[0] // P
    return np.ascontiguousarray(b.reshape(n, P).T)


def _run(x, w1e, b1, w2e, b2, wo, bo, d_in, d_h, d_out, n_cores=N_CORES, trace=False):
    b = x.shape[0]
    bc = b // n_cores
    kt1 = d_in // P

    nc = bacc.Bacc("TRN2", target_bir_lowering=False, debug=False, num_devices=n_cores)
    _build(nc, d_in, d_h, d_out, bc)

    np_bf16 = mybir.dt.np(BF16)

    def cvt(a):
        return np.ascontiguousarray(a.astype(np_bf16))

    shared = {
        "w1": cvt(_pack_lhsT(w1e, d_h, d_in)),
        "b1": _pack_bias(b1),
        "w2": cvt(_pack_lhsT(w2e, d_h, d_h)),
        "b2": _pack_bias(b2),
        "wo": cvt(_pack_out_panels(wo, d_out, d_h)),
        "bo": _pack_bias(bo),
    }
    in_maps = []
    for c in range(n_cores):
        xc = x[c * bc : (c + 1) * bc]  # [bc, d_in]
        # xt[kt][p, cc] = xc[cc, kt*128 + p]
        xt = np.ascontiguousarray(xc.T).reshape(kt1, P, bc)
        in_maps.append({"xt": cvt(xt), **shared})

    res = run_bass_kernel_spmd(nc, in_maps, core_ids=list(range(n_cores)), trace=trace)
    outs = []
    for c in range(n_cores):
        outs.append(res.results[c]["out"].reshape(d_out, bc))
    full = np.concatenate(outs, axis=1)  # [d_out, B]
    return np.ascontiguousarray(full.T), res


def kernel(x, W1, b1, W2, b2, Wo, bo, mask1, mask2):
    x = np.asarray(x, dtype=np.float32)
    w1e = np.asarray(W1, dtype=np.float32) * _expand_mask(mask1)
    w2e = np.asarray(W2, dtype=np.float32) * _expand_mask(mask2)
    out, _ = _run(
        x,
        w1e,
        np.asarray(b1, np.float32),
        w2e,
        np.asarray(b2, np.float32),
        np.asarray(Wo, np.float32),
        np.asarray(bo, np.float32),
        d_in=2048,
        d_h=4096,
        d_out=1024,
    )
    return out


# revision 31
# speedup vs baseline: 1.0089x; 1.0089x over previous
"""Block-sparse 3-layer MLP on 8 Trainium2 NeuronCores.

Reference computation (fp32):
    h1 = relu(x @ (W1*expand(mask1)).T + b1)       x:[B,2048] W1:[4096,2048]
    h2 = relu(h1 @ (W2*expand(mask2)).T + b2)      W2:[4096,4096]
    out = h2 @ Wo.T + bo                           Wo:[1024,4096] -> [B,1024]

Strategy: data-parallel over the batch (B=8192 -> 1024 rows/core), no
collectives. Masks are applied to the weights on the host (free) and all
matmuls run dense on the PE array: at density 0.5 with 32x32 mask tiles,
skipping zero tiles via PE-array tiling is slower than dense (packed 32x32
tiles reach only ~36% of dense PE throughput), and fp8 DoubleRow (2x PE
rate) fails the 2e-2 error gate (e4m3 one-pass ~6% rel err; an accurate
3-term hi/lo split needs 1.5x the products, i.e. slower than bf16).

All matmul operands are bf16: same 1 cycle/row PE rate as f32r but half
the DMA bytes and SBUF footprint (rel err ~4e-3, budget 2e-2).
Activations are feature-major [features, batch] so biases are
per-partition and lhsT panels are host-pretransposed [128, K] blocks.

Per core, three phases, PE-saturated throughout:
  L1: 32 m-tiles, psum-accumulated over 16 k-tiles, RELU -> h1 resident
      in SBUF (bf16, 8MB). Inputs stream on the three DMA-capable rings
      (sync/gpsimd/scalar) one descriptor per k-tile in PE consumption
      order, critical tiles at the ring heads.
  L2: 32 m-tiles over 32 k-tiles, RELU -> h2 resident (8MB). ps2 opens
      alongside ps1 (4+4 psum banks) so its first accumulation does not
      wait on L1's last psum release.
  L3: output accumulated directly in PSUM: 2 groups of 4 output m-tiles,
      each group's 4 accumulators [128,1024] = 8 psum banks, k-outer over
      the 32 h2 tiles. No SBUF accumulator and no vector adds; the tail
      is the last bias-activation (split scalar/vector) + output DMA
      spread over all three rings.
"""

import sys

sys.path.insert(0, "/opt/trn_rl_repo")

import numpy as np

from concourse import bacc, mybir, tile
from concourse.bass_utils import run_bass_kernel_spmd

F32 = mybir.dt.float32
BF16 = mybir.dt.bfloat16
RELU = mybir.ActivationFunctionType.Relu
IDENT = mybir.ActivationFunctionType.Identity

N_CORES = 8
TILE = 32  # block-sparse tile size of the masks
P = 128  # partitions


def _build(nc, d_in, d_h, d_out, bc):
    """Emit the per-core kernel. bc = batch columns per core."""
    kt1 = d_in // P  # k-tiles in layer 1 (16)
    mt1 = d_h // P  # m-tiles of h1 == k-tiles of layer 2 (32)
    mt2 = d_h // P  # m-tiles of h2 == k-tiles of layer 3 (32)
    mot = d_out // P  # m-tiles of out (8)
    sw = min(512, bc)  # psum strip width
    ns = bc // sw  # strips per row of tiles
    GW = 4  # output m-tiles per L3 psum group (4 x 2 banks = 8 banks)

    xt_d = nc.dram_tensor("xt", [kt1, P, bc], BF16, kind="ExternalInput")
    w1_d = nc.dram_tensor("w1", [mt1, P, d_in], BF16, kind="ExternalInput")
    b1_d = nc.dram_tensor("b1", [P, mt1], F32, kind="ExternalInput")
    w2_d = nc.dram_tensor("w2", [mt2, P, d_h], BF16, kind="ExternalInput")
    b2_d = nc.dram_tensor("b2", [P, mt2], F32, kind="ExternalInput")
    wo_d = nc.dram_tensor("wo", [mt2, P, d_out], BF16, kind="ExternalInput")
    bo_d = nc.dram_tensor("bo", [P, mot], F32, kind="ExternalInput")
    out_d = nc.dram_tensor("out", [mot, P, bc], F32, kind="ExternalOutput")

    with tile.TileContext(nc) as tc:
        with (
            tc.tile_pool(name="bias", bufs=1) as bias_pool,
            tc.tile_pool(name="h1", bufs=1) as h1_pool,
            tc.tile_pool(name="h2", bufs=1) as h2_pool,
            tc.tile_pool(name="w2p", bufs=3) as w2_pool,
        ):
            b1_sb = bias_pool.tile([P, mt1], F32, tag="b1")
            b2_sb = bias_pool.tile([P, mt2], F32, tag="b2")
            bo_sb = bias_pool.tile([P, mot], F32, tag="bo")

            h1 = []
            h2 = []
            wo_pre = {}
            if True:
                # ---------------- Layer 1 ----------------
                # ps1 and ps2 stay open across the L1->L2 boundary: closing
                # a psum pool at the boundary fences on the layer's LAST
                # RELU (the pool's final reader), stalling the PE ~1.2us.
                # Only the SBUF pools (last readers: the matmuls themselves)
                # close at the boundary. 4+4 banks; both close before ps3.
                es_ps1 = tc.tile_pool(name="ps1", bufs=2, space="PSUM")
                ps1_pool = es_ps1.__enter__()
                es_ps2 = tc.tile_pool(name="ps2", bufs=2, space="PSUM")
                ps2_pool = es_ps2.__enter__()
                with (
                    tc.tile_pool(name="xtp", bufs=1) as xt_pool,
                    tc.tile_pool(name="w1p", bufs=4) as w1_pool,
                ):
                    # The tile scheduler batches DMA-completion semaphores
                    # per ring segment between consumers, so loads must be
                    # EMITTED interleaved with the matmuls that consume them
                    # or the first matmul waits on a whole batch. Each xt
                    # k-tile is its own SBUF tile written by one descriptor
                    # (slices of a shared tile create cross-ring
                    # write-ordering semaphores); loads pace 3 k-tiles ahead
                    # of consumption, W1 panels 2 ahead, biases on the
                    # scalar ring where slack allows.
                    rings = [nc.sync, nc.gpsimd, nc.scalar]
                    xt = [
                        xt_pool.tile([P, bc], BF16, name=f"xt_{kt}", tag=f"xt_{kt}")
                        for kt in range(kt1)
                    ]

                    def load_xt(kt, eng):
                        eng.dma_start(out=xt[kt][:], in_=xt_d[kt])

                    # critical head on the two HW-DGE rings (sync, scalar);
                    # gpsimd's SW-DGE completion signals are coalesced and
                    # release late, so it only carries loads with slack
                    load_xt(0, nc.scalar)
                    w1pre = {}
                    w2pre = []
                    w1t0 = w1_pool.tile([P, d_in], BF16, tag="w1t")
                    nc.sync.dma_start(out=w1t0[:], in_=w1_d[0])
                    w1pre[0] = w1t0
                    load_xt(1, nc.sync)
                    nc.scalar.dma_start(out=b1_sb[:], in_=b1_d[:])
                    load_xt(2, nc.scalar)
                    load_xt(3, nc.sync)
                    xt_next = 4
                    w1_next = 1

                    def issue_w1(mt):
                        t = w1_pool.tile([P, d_in], BF16, tag="w1t")
                        eng = nc.sync if mt == 1 else rings[mt % 3]
                        eng.dma_start(out=t[:], in_=w1_d[mt])
                        w1pre[mt] = t

                    for mt in range(mt1):
                        while mt > 0 and w1_next <= min(mt + 2, mt1 - 1):
                            issue_w1(w1_next)
                            w1_next += 1
                        w1t = w1pre.pop(mt)
                        ps = ps1_pool.tile([P, bc], F32, tag="ps1")
                        for kt in range(kt1):
                            if mt == 0:
                                while xt_next <= min(kt + 3, kt1 - 1):
                                    if xt_next < 8:
                                        eng = nc.sync if xt_next % 2 else nc.scalar
                                    else:
                                        eng = rings[xt_next % 3]
                                    load_xt(xt_next, eng)
                                    xt_next += 1
                                if kt == 6:
                                    # W1 panels 1,2 once xt's head is in
                                    while w1_next <= 2:
                                        issue_w1(w1_next)
                                        w1_next += 1
                            for n in range(ns):
                                nc.tensor.matmul(
                                    ps[:, n * sw : (n + 1) * sw],
                                    w1t[:, kt * P : (kt + 1) * P],
                                    xt[kt][:, n * sw : (n + 1) * sw],
                                    start=(kt == 0),
                                    stop=(kt == kt1 - 1),
                                )
                        if mt == 0:
                            nc.scalar.dma_start(out=b2_sb[:], in_=b2_d[:])
                            nc.scalar.dma_start(out=bo_sb[:], in_=bo_d[:])
                        if mt == 3:
                            # W2 panels 0,1 on the now-idle scalar ring
                            for i in range(2):
                                t = w2_pool.tile([P, d_h], BF16, tag="w2t")
                                nc.scalar.dma_start(out=t[:], in_=w2_d[i])
                                w2pre.append(t)
                        h = h1_pool.tile(
                            [P, bc], BF16, name=f"h1_{mt}", tag=f"h1_{mt}"
                        )
                        nc.scalar.activation(
                            h[:], ps[:], RELU, bias=b1_sb[:, mt : mt + 1]
                        )
                        h1.append(h)

                # ---------------- Layer 2 ----------------
                es_wop = tc.tile_pool(name="wop", bufs=10)
                wo_pool = es_wop.__enter__()
                w2map = {0: w2pre[0], 1: w2pre[1]}
                w2_next = 2

                def issue_w2(mt):
                    t = w2_pool.tile([P, d_h], BF16, tag="w2t")
                    eng = nc.sync if mt % 2 else nc.gpsimd
                    eng.dma_start(out=t[:], in_=w2_d[mt])
                    w2map[mt] = t

                for mt in range(mt2):
                    while w2_next <= min(mt + 1, mt2 - 1):
                        issue_w2(w2_next)
                        w2_next += 1
                    w2t = w2map.pop(mt)
                    ps = ps2_pool.tile([P, bc], F32, tag="ps2")
                    for kt in range(mt1):
                        for n in range(ns):
                            nc.tensor.matmul(
                                ps[:, n * sw : (n + 1) * sw],
                                w2t[:, kt * P : (kt + 1) * P],
                                h1[kt][:, n * sw : (n + 1) * sw],
                                start=(kt == 0),
                                stop=(kt == mt1 - 1),
                            )
                    h = h2_pool.tile([P, bc], BF16, name=f"h2_{mt}", tag=f"h2_{mt}")
                    nc.scalar.activation(h[:], ps[:], RELU, bias=b2_sb[:, mt : mt + 1])
                    h2.append(h)
                    if mt == 24:
                        # L3's first half-panels of Wo on the mostly-idle
                        # scalar ring, well ahead of L3's start
                        for kt in range(4):
                            t = wo_pool.tile([P, GW * P], BF16, tag="wot")
                            nc.scalar.dma_start(out=t[:], in_=wo_d[kt][:, 0 : GW * P])
                            wo_pre[kt] = t

            es_ps2.__exit__(None, None, None)
            es_ps1.__exit__(None, None, None)

            # ---------------- Layer 3 ----------------
            # Output accumulates in PSUM across all 32 k-tiles: per group
            # of GW=4 output m-tiles, 4 accumulators of [128, bc] f32
            # occupy all 8 psum banks; Wo half-panels stream k-outer.
            with (
                tc.tile_pool(name="ps3", bufs=1, space="PSUM") as ps3_pool,
                tc.tile_pool(name="osb", bufs=GW) as osb_pool,
            ):
                seq = [(g, kt) for g in range(mot // GW) for kt in range(mt2)]
                wot_map = {(0, kt): t for kt, t in wo_pre.items()}
                wo_next = 0

                def issue_wo(g, kt):
                    if (g, kt) in wot_map:
                        return
                    t = wo_pool.tile([P, GW * P], BF16, tag="wot")
                    eng = nc.sync if kt % 2 else nc.gpsimd
                    eng.dma_start(
                        out=t[:], in_=wo_d[kt][:, g * GW * P : (g + 1) * GW * P]
                    )
                    wot_map[(g, kt)] = t

                for idx, (g, kt) in enumerate(seq):
                    if kt == 0:
                        pss = [
                            ps3_pool.tile(
                                [P, bc], F32, name=f"ps3_{j}", tag=f"ps3_{j}"
                            )
                            for j in range(GW)
                        ]
                    while wo_next <= min(idx + 4, len(seq) - 1):
                        issue_wo(*seq[wo_next])
                        wo_next += 1
                    wot = wot_map.pop((g, kt))
                    for j in range(GW):
                        for n in range(ns):
                            nc.tensor.matmul(
                                pss[j][:, n * sw : (n + 1) * sw],
                                wot[:, j * P : (j + 1) * P],
                                h2[kt][:, n * sw : (n + 1) * sw],
                                start=(kt == 0),
                                stop=(kt == mt2 - 1),
                            )
                    if kt != mt2 - 1:
                        continue
                    last_g = g == mot // GW - 1
                    # finalize all accumulators first (scalar half / vector
                    # half per j), then issue the output DMAs on the two
                    # HW-DGE rings only -- gpsimd SW-DGE copies would hold
                    # the final drain hostage, and DMA issues interleaved on
                    # the scalar queue would delay the activations.
                    osbs = []
                    for j in range(GW):
                        mo = g * GW + j
                        osb = osb_pool.tile([P, bc], F32, tag="osb")
                        if j == 0 and not last_g:
                            # single full-width op releases this psum
                            # buffer fastest for the next group
                            nc.scalar.activation(
                                osb[:], pss[j][:], IDENT, bias=bo_sb[:, mo : mo + 1]
                            )
                        else:
                            nc.scalar.activation(
                                osb[:, 0:sw],
                                pss[j][:, 0:sw],
                                IDENT,
                                bias=bo_sb[:, mo : mo + 1],
                            )
                            nc.vector.tensor_scalar_add(
                                osb[:, sw:], pss[j][:, sw:], bo_sb[:, mo : mo + 1]
                            )
                        osbs.append(osb)
                    for j in range(GW):
                        mo = g * GW + j
                        osb = osbs[j]
                        nc.sync.dma_start(out=out_d[mo][:, 0:sw], in_=osb[:, 0:sw])
                        nc.scalar.dma_start(out=out_d[mo][:, sw:], in_=osb[:, sw:])

            es_wop.__exit__(None, None, None)

    nc.compile()
    return nc


def _expand_mask(mask, t=TILE):
    return np.repeat(np.repeat(np.asarray(mask, dtype=bool), t, axis=0), t, axis=1)


def _pack_lhsT(w, d_m, d_k):
    """[d_m, d_k] weights -> [d_m/P, P, d_k] panels.

    panel[mt, i, kt*P + j] = w[mt*P + j, kt*P + i], so each [P, P] slice of a
    panel is a ready-to-use lhsT block (partition dim = contraction dim).
    """
    mt, kt = d_m // P, d_k // P
    return np.ascontiguousarray(
        w.reshape(mt, P, kt, P).transpose(0, 3, 2, 1).reshape(mt, P, d_k)
    )


def _pack_out_panels(w, d_m, d_k):
    """[d_m, d_k] weights -> [d_k/P, P, d_m] panels keyed by the k-tile.

    panel[kt, i, mo*P + j] = w[mo*P + j, kt*P + i].
    """
    mt, kt = d_m // P, d_k // P
    return np.ascontiguousarray(
        w.reshape(mt, P, kt, P).transpose(2, 3, 0, 1).reshape(kt, P, d_m)
    )


def _pack_bias(b):
    n = b.shape[0] // P
    return np.ascontiguousarray(b.reshape(n, P).T)


def _run(x, w1e, b1, w2e, b2, wo, bo, d_in, d_h, d_out, n_cores=N_CORES, trace=False):
    b = x.shape[0]
    bc = b // n_cores
    kt1 = d_in // P

    nc = bacc.Bacc("TRN2", target_bir_lowering=False, debug=False, num_devices=n_cores)
    _build(nc, d_in, d_h, d_out, bc)

    np_bf16 = mybir.dt.np(BF16)

    def cvt(a):
        return np.ascontiguousarray(a.astype(np_bf16))

    shared = {
        "w1": cvt(_pack_lhsT(w1e, d_h, d_in)),
        "b1": _pack_bias(b1),
        "w2": cvt(_pack_lhsT(w2e, d_h, d_h)),
        "b2": _pack_bias(b2),
        "wo": cvt(_pack_out_panels(wo, d_out, d_h)),
        "bo": _pack_bias(bo),
    }
    in_maps = []
    for c in range(n_cores):
        xc = x[c * bc : (c + 1) * bc]  # [bc, d_in]
        # xt[kt][p, cc] = xc[cc, kt*128 + p]
        xt = np.ascontiguousarray(xc.T).reshape(kt1, P, bc)
        in_maps.append({"xt": cvt(xt), **shared})

    res = run_bass_kernel_spmd(nc, in_maps, core_ids=list(range(n_cores)), trace=trace)
    outs = []
    for c in range(n_cores):
        outs.append(res.results[c]["out"].reshape(d_out, bc))
    full = np.concatenate(outs, axis=1)  # [d_out, B]
    return np.ascontiguousarray(full.T), res


def kernel(x, W1, b1, W2, b2, Wo, bo, mask1, mask2):
    x = np.asarray(x, dtype=np.float32)
    w1e = np.asarray(W1, dtype=np.float32) * _expand_mask(mask1)
    w2e = np.asarray(W2, dtype=np.float32) * _expand_mask(mask2)
    out, _ = _run(
        x,
        w1e,
        np.asarray(b1, np.float32),
        w2e,
        np.asarray(b2, np.float32),
        np.asarray(Wo, np.float32),
        np.asarray(bo, np.float32),
        d_in=2048,
        d_h=4096,
        d_out=1024,
    )
    return out
